# revision 15
# baseline (speedup 1.0000x reference)
"""FP8 quantized matmul kernel for Trainium2 (8 NeuronCores, SPMD).

Computes: out = fp8_quant(input) @ fp8_quant(other), bf16 output.
  input: [16384, 2048] fp32, other: [2048, 2048] fp32.

Sharding: data-parallel over M. Each core processes 2048 rows of `input`
and a full replica of `other`; no cross-core communication. During
host-side sharding both operands are packed K-major into 512-wide
panel-of-column blocks ([128ki, panel, ko, 512] fp32), so every device
load is per-partition contiguous (8 KB lines, peak HBM efficiency) and
no on-device transposes are needed.

Per-core pipeline (all on device):
  1. A panels (input^T columns) and B panels (other columns) stream in
     as [128, 4ko, 512] chunks via gpsimd SWDGE DMAs that cast
     fp32 -> fp8e4m3 in flight (RNE saturating, exactly matching the
     reference quant for ~N(0,1) data where the +-448 clip never fires)
     straight into SBUF-resident qat / qb: no stage buffers and no
     Vector/Scalar quant work (lower power -> fewer PE util-throttle
     windows). Streaming is B-ahead ((A0|B0|B1), (B2|A1), (B3|A2), (A3))
     so the last-arriving panel gates only 4 output tiles.
  2. FP8 DoubleRow matmuls (K paired 2x128) accumulate fp32 in PSUM;
     tiles are emitted the moment their last operand's load is issued so
     the in-order PE queue matches data arrival (first MM at ~13us of a
     ~158us kernel; PE runs at the 222 ns/MM DoubleRow peak after ramp).
  3. PSUM evicts to bf16 on Vector (3 of 4 slices) / Scalar (last slice)
     and stores via the Scalar-engine HWDGE queue (separate from load
     issue so store waits never block loads), batched [128, 4, 512].

Measured (8-core SPMD, axon trn2, NTFF device exec time, core 0):
~158 us in unthrottled windows (~170 us when the chip power-throttles
the PE; ham type-1 k=4/n=8 windows). Baseline this session started at
~192 us (on-device PE transposes + whole-matrix-first schedule).
"""

import numpy as np

P = 128
M_LOC, K, N = 2048, 2048, 2048
N_CORES = 8
KO = K // P       # 16 k-blocks of 128
KP = KO // 2      # 8 DoubleRow k-pairs
FD = 512          # matmul free dim (one PSUM bank of fp32)
NT = N // FD      # 4 n panels
MG = M_LOC // FD  # 4 m groups (512 wide)
MI = FD // P      # 4 m slices per group
CKO = 4           # ko blocks per streamed chunk
KC = KO // CKO    # 4 k-chunks per panel/group

import os
XF_BUFS = int(os.environ.get('XF_BUFS', '3'))
WF_BUFS = int(os.environ.get('WF_BUFS', '3'))
OSB_BUFS = int(os.environ.get('OSB_BUFS', '4'))
PSUM_BUFS = int(os.environ.get('PSUM_BUFS', '8'))
OUT_ENG = os.environ.get('OUT_ENG', 'scalar')  # scalar | gpsimd | sync


def build(tc, xp, wp, out, iters=1, hw_loop=False):
    """Emit the per-core kernel IR. xp: [128, MG, KO, FD] f32 (the input
    shard, K-major panel-packed), wp: [128, NT, KO, FD] f32 (other,
    panel-packed), out: [M_LOC,N] bf16 (all DRAM APs). iters>1 repeats
    the whole computation (python-unrolled, or a hardware For_i loop when
    hw_loop=True) for marginal-time benchmarking."""
    import contextlib

    import concourse.mybir as mybir

    nc = tc.nc
    f32 = mybir.dt.float32
    bf16 = mybir.dt.bfloat16
    fp8 = mybir.dt.float8e4

    out_r = out.rearrange("(t p) n -> p t n", p=P)  # m row = t*128 + p

    with (
        tc.tile_pool(name="resident", bufs=1) as resident,
        tc.tile_pool(name="stage", bufs=4) as stage,
        tc.tile_pool(name="ostage", bufs=4) as ostage,
        tc.tile_pool(name="psum_mm", bufs=PSUM_BUFS, space="PSUM") as psum_mm,
    ):
        if hw_loop:
            loop_ctx = tc.For_i(0, iters, 1)
            reps = 1
        else:
            loop_ctx = contextlib.nullcontext()
            reps = iters

        with loop_ctx:
            for _ in range(reps):
                _emit_body(tc, xp, wp, out_r, resident, stage, ostage,
                           psum_mm, mybir, f32, bf16, fp8)


def _emit_body(tc, xp, wp, out_r, resident, stage, ostage, psum_mm,
               mybir, f32, bf16, fp8):
    nc = tc.nc

    # [ki, g, ko, m] = quant(input)^T at k = ko*128 + ki, m = g*512 + m
    qat = resident.tile([P, MG, KO, FD], fp8, tag="qat")
    # [ki, p, ko, n] = quant(other) at k = ko*128 + ki, n = p*512 + n
    qb = resident.tile([P, NT, KO, FD], fp8, tag="qb")

    QCAST = os.environ.get('QCAST', 'xw')  # chars 'x'/'w': quantize via SWDGE DMA cast

    def load_quant_chunk(src, dst, col, ko0, nko, which):
        """DMA one [128, nko, FD] f32 chunk (ko blocks [ko0, ko0+nko) of
        panel `col`, per-partition contiguous in DRAM) and quantize it
        into the fp8 resident tile. Staged in CKO-sized slots so small
        head chunks and full chunks share one pool tag."""
        ks = slice(ko0, ko0 + nko)
        if which in QCAST:
            # gpsimd SWDGE casts f32 -> fp8 in flight: no stage buffer,
            # no quant engine work.
            nc.gpsimd.dma_start(dst[:, col, ks, :], src[:, col, ks, :])
            return
        f = stage.tile(
            [P, CKO, FD], f32, tag=f"{which}f", name=f"{which}f_{col}_{ko0}",
            bufs=(XF_BUFS if which == "x" else WF_BUFS),
        )
        nc.sync.dma_start(f[:, :nko, :], src[:, col, ks, :])
        eng = nc.vector.tensor_copy if which == "x" else nc.scalar.copy
        eng(dst[:, col, ks, :], f[:, :nko, :])

    def mm_tile(g, p):
        """All MMs for output tile (m-group g, n-panel p): 4 m-slices of
        [128, 512], each accumulating 8 DoubleRow k-pairs in PSUM."""
        osb = ostage.tile(
            [P, MI, FD], bf16, tag="osb", name=f"osb_{g}_{p}", bufs=OSB_BUFS
        )
        for mi in range(MI):
            ps = psum_mm.tile([P, FD], f32, tag="ps", name=f"ps_{g}_{p}_{mi}")
            for kp in range(KP):
                nc.tensor.matmul(
                    ps,
                    qat[:, g, 2 * kp : 2 * kp + 2, mi * P : (mi + 1) * P],
                    qb[:, p, 2 * kp : 2 * kp + 2, :],
                    start=(kp == 0),
                    stop=(kp == KP - 1),
                    perf_mode=mybir.MatmulPerfMode.DoubleRow,
                )
            if mi % 2 == 1:
                nc.scalar.copy(osb[:, mi, :], ps)
            else:
                nc.vector.tensor_copy(osb[:, mi, :], ps)
        out_eng = {"scalar": nc.scalar, "gpsimd": nc.gpsimd, "sync": nc.sync}[OUT_ENG]
        out_eng.dma_start(
            out_r[:, g * MI : (g + 1) * MI, p * FD : (p + 1) * FD], osb
        )

    # k-chunk splits: small head chunks for the first pair so the tensor
    # engine starts as early as possible.
    HEAD = [(0, 2), (2, 2), (4, 4), (8, 4), (12, 4)]
    FINE = [(0, 2), (2, 2), (4, 2), (6, 2), (8, 2), (10, 2), (12, 2), (14, 2)]
    FULL = [(0, 4), (4, 4), (8, 4), (12, 4)]

    # B-ahead streaming: phase 1 round-robins (A0 | B0 | B1) so two B
    # panels land by the time A0's tiles are consumed; later phases keep
    # >=2 streams chunk-interleaved (single-stream issue loses ~20% DMA
    # bandwidth to stage-buffer stalls on the in-order sync queue).
    # Tiles are emitted the moment their last operand's load has been
    # issued, so the PE queue order matches data arrival. The
    # last-arriving panel (A3) gates only 4 tiles of PE work.
    SCHED = os.environ.get('SCHED', 'a')
    if SCHED == 'a':
        phases = [
            ([("x", 0, HEAD), ("w", 0, HEAD), ("w", 1, FULL)],
             [(0, 0), (0, 1)]),
            ([("w", 2, FULL), ("x", 1, FULL)],
             [(0, 2), (1, 0), (1, 1), (1, 2)]),
            ([("w", 3, FULL), ("x", 2, FULL)],
             [(0, 3), (1, 3), (2, 0), (2, 1), (2, 2), (2, 3)]),
            ([("x", 3, FULL)],
             [(3, 0), (3, 1), (3, 2), (3, 3)]),
        ]
    elif SCHED == 'b':  # 4-way phase 1
        phases = [
            ([("x", 0, HEAD), ("w", 0, HEAD), ("w", 1, FULL), ("w", 2, FULL)],
             [(0, 0), (0, 1), (0, 2)]),
            ([("w", 3, FULL), ("x", 1, FULL)],
             [(0, 3), (1, 0), (1, 1), (1, 2), (1, 3)]),
            ([("x", 2, FULL), ("x", 3, FULL)],
             [(2, 0), (2, 1), (2, 2), (2, 3), (3, 0), (3, 1), (3, 2), (3, 3)]),
        ]
    elif SCHED == 'd':  # fine chunks everywhere
        phases = [
            ([("x", 0, FINE), ("w", 0, FINE), ("w", 1, FINE)],
             [(0, 0), (0, 1)]),
            ([("w", 2, FINE), ("x", 1, FINE)],
             [(0, 2), (1, 0), (1, 1), (1, 2)]),
            ([("w", 3, FINE), ("x", 2, FINE)],
             [(0, 3), (1, 3), (2, 0), (2, 1), (2, 2), (2, 3)]),
            ([("x", 3, FINE)],
             [(3, 0), (3, 1), (3, 2), (3, 3)]),
        ]
    else:  # 'c': fine chunks for pair 0
        phases = [
            ([("x", 0, FINE), ("w", 0, FINE), ("w", 1, FULL)],
             [(0, 0), (0, 1)]),
            ([("w", 2, FULL), ("x", 1, FULL)],
             [(0, 2), (1, 0), (1, 1), (1, 2)]),
            ([("w", 3, FULL), ("x", 2, FULL)],
             [(0, 3), (1, 3), (2, 0), (2, 1), (2, 2), (2, 3)]),
            ([("x", 3, FULL)],
             [(3, 0), (3, 1), (3, 2), (3, 3)]),
        ]
    for loads, tiles in phases:
        nchunks = max(len(cl) for _, _, cl in loads)
        for i in range(nchunks):
            for which, panel, chunklist in loads:
                if i < len(chunklist):
                    k0, nko = chunklist[i]
                    load_quant_chunk(
                        xp if which == "x" else wp,
                        qat if which == "x" else qb,
                        panel, k0, nko, which,
                    )
        for g, p in tiles:
            mm_tile(g, p)


def build_program(iters=1):
    """Build and compile the single-core SPMD program."""
    import concourse.bacc as bacc
    import concourse.mybir as mybir
    import concourse.tile as tile

    nc = bacc.Bacc("TRN2", target_bir_lowering=False, debug=False)
    xp = nc.dram_tensor(
        "xp", [P, MG, KO, FD], mybir.dt.float32, kind="ExternalInput"
    ).ap()
    wp = nc.dram_tensor(
        "wp", [P, NT, KO, FD], mybir.dt.float32, kind="ExternalInput"
    ).ap()
    out = nc.dram_tensor(
        "out", [M_LOC, N], mybir.dt.bfloat16, kind="ExternalOutput"
    ).ap()
    with tile.TileContext(nc) as tc:
        build(tc, xp, wp, out, iters=iters)
    nc.compile()
    return nc


_PROGRAM_CACHE = {}


def _pack_panels(a_t_like):
    """[K, C] fp32 -> [128ki, C/512 panel, 16ko, 512] (k = ko*128 + ki)."""
    return np.ascontiguousarray(
        a_t_like.reshape(KO, P, -1, FD).transpose(1, 2, 0, 3)
    )


def make_in_maps(input, other):
    input = np.asarray(input, dtype=np.float32)
    other = np.asarray(other, dtype=np.float32)
    wp = _pack_panels(other)
    return [
        {
            "xp": _pack_panels(input[c * M_LOC : (c + 1) * M_LOC].T),
            "wp": wp,
        }
        for c in range(N_CORES)
    ]


def kernel(input, other):
    from concourse.bass_utils import run_bass_kernel_spmd

    if "nc" not in _PROGRAM_CACHE:
        _PROGRAM_CACHE["nc"] = build_program()
    nc = _PROGRAM_CACHE["nc"]

    in_maps = make_in_maps(input, other)
    res = run_bass_kernel_spmd(nc, in_maps, list(range(N_CORES)))
    return np.concatenate([res.results[c]["out"] for c in range(N_CORES)], axis=0)


# revision 16
# speedup vs baseline: 1.1114x; 1.1114x over previous
"""FP8 quantized matmul kernel for Trainium2 (8 NeuronCores, SPMD).

Computes: out = fp8_quant(input) @ fp8_quant(other), bf16 output.
  input: [16384, 2048] fp32, other: [2048, 2048] fp32.

Sharding: data-parallel over M. Each core processes 2048 rows of `input`
and a full replica of `other`; no cross-core communication. During
host-side sharding both operands are packed K-major into 512-wide
panel-of-column blocks ([128ki, panel, ko, 512] fp32), so every device
load is per-partition contiguous (8 KB lines, peak HBM efficiency) and
no on-device transposes are needed.

Per-core pipeline (all on device):
  1. A panels (input^T columns) and B panels (other columns) stream in
     as [128, 4ko, 512] chunks via gpsimd SWDGE DMAs that cast
     fp32 -> fp8e4m3 in flight (RNE saturating, exactly matching the
     reference quant for ~N(0,1) data where the +-448 clip never fires)
     straight into SBUF-resident qat / qb: no stage buffers and no
     Vector/Scalar quant work (lower power -> fewer PE util-throttle
     windows). Streaming is B-ahead ((A0|B0|B1), (B2|A1), (B3|A2), (A3))
     so the last-arriving panel gates only 4 output tiles.
  2. FP8 DoubleRow matmuls (K paired 2x128) accumulate fp32 in PSUM;
     tiles are emitted the moment their last operand's load is issued so
     the in-order PE queue matches data arrival (first MM at ~13us of a
     ~158us kernel; PE runs at the 222 ns/MM DoubleRow peak after ramp).
  3. PSUM evicts to bf16 on Vector (3 of 4 slices) / Scalar (last slice)
     and stores via the Scalar-engine HWDGE queue (separate from load
     issue so store waits never block loads), batched [128, 4, 512].

Measured (8-core SPMD, axon trn2, NTFF device exec time, core 0):
~158 us in unthrottled windows (~170 us when the chip power-throttles
the PE; ham type-1 k=4/n=8 windows). Baseline this session started at
~192 us (on-device PE transposes + whole-matrix-first schedule).
"""

import numpy as np

P = 128
M_LOC, K, N = 2048, 2048, 2048
N_CORES = 8
KO = K // P       # 16 k-blocks of 128
KP = KO // 2      # 8 DoubleRow k-pairs
FD = 512          # matmul free dim (one PSUM bank of fp32)
NT = N // FD      # 4 n panels
MG = M_LOC // FD  # 4 m groups (512 wide)
MI = FD // P      # 4 m slices per group
CKO = 4           # ko blocks per streamed chunk
KC = KO // CKO    # 4 k-chunks per panel/group

import os
XF_BUFS = int(os.environ.get('XF_BUFS', '3'))
WF_BUFS = int(os.environ.get('WF_BUFS', '3'))
OSB_BUFS = int(os.environ.get('OSB_BUFS', '4'))
PSUM_BUFS = int(os.environ.get('PSUM_BUFS', '8'))
OUT_ENG = os.environ.get('OUT_ENG', 'scalar')  # scalar | gpsimd | sync


def build(tc, xp, wp, out, iters=1, hw_loop=False):
    """Emit the per-core kernel IR. xp: [128, MG, KO, FD] f32 (the input
    shard, K-major panel-packed), wp: [128, NT, KO, FD] f32 (other,
    panel-packed), out: [M_LOC,N] bf16 (all DRAM APs). iters>1 repeats
    the whole computation (python-unrolled, or a hardware For_i loop when
    hw_loop=True) for marginal-time benchmarking."""
    import contextlib

    import concourse.mybir as mybir

    nc = tc.nc
    f32 = mybir.dt.float32
    bf16 = mybir.dt.bfloat16
    fp8 = mybir.dt.float8e4

    out_r = out.rearrange("(t p) n -> p t n", p=P)  # m row = t*128 + p

    with (
        tc.tile_pool(name="resident", bufs=1) as resident,
        tc.tile_pool(name="stage", bufs=4) as stage,
        tc.tile_pool(name="ostage", bufs=4) as ostage,
        tc.tile_pool(name="psum_mm", bufs=PSUM_BUFS, space="PSUM") as psum_mm,
    ):
        if hw_loop:
            loop_ctx = tc.For_i(0, iters, 1)
            reps = 1
        else:
            loop_ctx = contextlib.nullcontext()
            reps = iters

        with loop_ctx:
            for _ in range(reps):
                _emit_body(tc, xp, wp, out_r, resident, stage, ostage,
                           psum_mm, mybir, f32, bf16, fp8)


def _emit_body(tc, xp, wp, out_r, resident, stage, ostage, psum_mm,
               mybir, f32, bf16, fp8):
    nc = tc.nc

    # [ki, g, ko, m] = quant(input)^T at k = ko*128 + ki, m = g*512 + m
    qat = resident.tile([P, MG, KO, FD], fp8, tag="qat")
    # [ki, p, ko, n] = quant(other) at k = ko*128 + ki, n = p*512 + n
    qb = resident.tile([P, NT, KO, FD], fp8, tag="qb")

    QCAST = os.environ.get('QCAST', 'xw')  # chars 'x'/'w': quantize via SWDGE DMA cast

    def load_quant_chunk(src, dst, col, ko0, nko, which):
        """DMA one [128, nko, FD] f32 chunk (ko blocks [ko0, ko0+nko) of
        panel `col`, per-partition contiguous in DRAM) and quantize it
        into the fp8 resident tile. Staged in CKO-sized slots so small
        head chunks and full chunks share one pool tag."""
        ks = slice(ko0, ko0 + nko)
        if which in QCAST:
            # gpsimd SWDGE casts f32 -> fp8 in flight: no stage buffer,
            # no quant engine work.
            nc.gpsimd.dma_start(dst[:, col, ks, :], src[:, col, ks, :])
            return
        f = stage.tile(
            [P, CKO, FD], f32, tag=f"{which}f", name=f"{which}f_{col}_{ko0}",
            bufs=(XF_BUFS if which == "x" else WF_BUFS),
        )
        nc.sync.dma_start(f[:, :nko, :], src[:, col, ks, :])
        eng = nc.vector.tensor_copy if which == "x" else nc.scalar.copy
        eng(dst[:, col, ks, :], f[:, :nko, :])

    def mm_tile(g, p):
        """All MMs for output tile (m-group g, n-panel p): 4 m-slices of
        [128, 512], each accumulating 8 DoubleRow k-pairs in PSUM."""
        osb = ostage.tile(
            [P, MI, FD], bf16, tag="osb", name=f"osb_{g}_{p}", bufs=OSB_BUFS
        )
        for mi in range(MI):
            ps = psum_mm.tile([P, FD], f32, tag="ps", name=f"ps_{g}_{p}_{mi}")
            for kp in range(KP):
                nc.tensor.matmul(
                    ps,
                    qat[:, g, 2 * kp : 2 * kp + 2, mi * P : (mi + 1) * P],
                    qb[:, p, 2 * kp : 2 * kp + 2, :],
                    start=(kp == 0),
                    stop=(kp == KP - 1),
                    perf_mode=mybir.MatmulPerfMode.DoubleRow,
                )
            if mi % 2 == 1:
                nc.scalar.copy(osb[:, mi, :], ps)
            else:
                nc.vector.tensor_copy(osb[:, mi, :], ps)
        out_eng = {"scalar": nc.scalar, "gpsimd": nc.gpsimd, "sync": nc.sync}[OUT_ENG]
        out_eng.dma_start(
            out_r[:, g * MI : (g + 1) * MI, p * FD : (p + 1) * FD], osb
        )

    # k-chunk splits: small head chunks for the first pair so the tensor
    # engine starts as early as possible.
    HEAD = [(0, 2), (2, 2), (4, 4), (8, 4), (12, 4)]
    FINE = [(0, 2), (2, 2), (4, 2), (6, 2), (8, 2), (10, 2), (12, 2), (14, 2)]
    FULL = [(0, 4), (4, 4), (8, 4), (12, 4)]

    # B-ahead streaming: phase 1 round-robins (A0 | B0 | B1) so two B
    # panels land by the time A0's tiles are consumed; later phases keep
    # >=2 streams chunk-interleaved (single-stream issue loses ~20% DMA
    # bandwidth to stage-buffer stalls on the in-order sync queue).
    # Tiles are emitted the moment their last operand's load has been
    # issued, so the PE queue order matches data arrival. The
    # last-arriving panel (A3) gates only 4 tiles of PE work.
    SCHED = os.environ.get('SCHED', 'a')
    if SCHED == 'a':
        phases = [
            ([("x", 0, HEAD), ("w", 0, HEAD), ("w", 1, FULL)],
             [(0, 0), (0, 1)]),
            ([("w", 2, FULL), ("x", 1, FULL)],
             [(0, 2), (1, 0), (1, 1), (1, 2)]),
            ([("w", 3, FULL), ("x", 2, FULL)],
             [(0, 3), (1, 3), (2, 0), (2, 1), (2, 2), (2, 3)]),
            ([("x", 3, FULL)],
             [(3, 0), (3, 1), (3, 2), (3, 3)]),
        ]
    elif SCHED == 'b':  # 4-way phase 1
        phases = [
            ([("x", 0, HEAD), ("w", 0, HEAD), ("w", 1, FULL), ("w", 2, FULL)],
             [(0, 0), (0, 1), (0, 2)]),
            ([("w", 3, FULL), ("x", 1, FULL)],
             [(0, 3), (1, 0), (1, 1), (1, 2), (1, 3)]),
            ([("x", 2, FULL), ("x", 3, FULL)],
             [(2, 0), (2, 1), (2, 2), (2, 3), (3, 0), (3, 1), (3, 2), (3, 3)]),
        ]
    elif SCHED == 'e':  # 2-way prime of the first chunks, then B-ahead
        phases = [
            ([("x", 0, HEAD[:1]), ("w", 0, HEAD[:1])], []),
            ([("x", 0, HEAD[1:]), ("w", 0, HEAD[1:]), ("w", 1, FULL)],
             [(0, 0), (0, 1)]),
            ([("w", 2, FULL), ("x", 1, FULL)],
             [(0, 2), (1, 0), (1, 1), (1, 2)]),
            ([("w", 3, FULL), ("x", 2, FULL)],
             [(0, 3), (1, 3), (2, 0), (2, 1), (2, 2), (2, 3)]),
            ([("x", 3, FULL)],
             [(3, 0), (3, 1), (3, 2), (3, 3)]),
        ]
    elif SCHED == 'd':  # fine chunks everywhere
        phases = [
            ([("x", 0, FINE), ("w", 0, FINE), ("w", 1, FINE)],
             [(0, 0), (0, 1)]),
            ([("w", 2, FINE), ("x", 1, FINE)],
             [(0, 2), (1, 0), (1, 1), (1, 2)]),
            ([("w", 3, FINE), ("x", 2, FINE)],
             [(0, 3), (1, 3), (2, 0), (2, 1), (2, 2), (2, 3)]),
            ([("x", 3, FINE)],
             [(3, 0), (3, 1), (3, 2), (3, 3)]),
        ]
    else:  # 'c': fine chunks for pair 0
        phases = [
            ([("x", 0, FINE), ("w", 0, FINE), ("w", 1, FULL)],
             [(0, 0), (0, 1)]),
            ([("w", 2, FULL), ("x", 1, FULL)],
             [(0, 2), (1, 0), (1, 1), (1, 2)]),
            ([("w", 3, FULL), ("x", 2, FULL)],
             [(0, 3), (1, 3), (2, 0), (2, 1), (2, 2), (2, 3)]),
            ([("x", 3, FULL)],
             [(3, 0), (3, 1), (3, 2), (3, 3)]),
        ]
    for loads, tiles in phases:
        nchunks = max(len(cl) for _, _, cl in loads)
        for i in range(nchunks):
            for which, panel, chunklist in loads:
                if i < len(chunklist):
                    k0, nko = chunklist[i]
                    load_quant_chunk(
                        xp if which == "x" else wp,
                        qat if which == "x" else qb,
                        panel, k0, nko, which,
                    )
        for g, p in tiles:
            mm_tile(g, p)


def build_program(iters=1):
    """Build and compile the single-core SPMD program."""
    import concourse.bacc as bacc
    import concourse.mybir as mybir
    import concourse.tile as tile

    nc = bacc.Bacc("TRN2", target_bir_lowering=False, debug=False)
    xp = nc.dram_tensor(
        "xp", [P, MG, KO, FD], mybir.dt.float32, kind="ExternalInput"
    ).ap()
    wp = nc.dram_tensor(
        "wp", [P, NT, KO, FD], mybir.dt.float32, kind="ExternalInput"
    ).ap()
    out = nc.dram_tensor(
        "out", [M_LOC, N], mybir.dt.bfloat16, kind="ExternalOutput"
    ).ap()
    with tile.TileContext(nc) as tc:
        build(tc, xp, wp, out, iters=iters)
    nc.compile()
    return nc


_PROGRAM_CACHE = {}


def _pack_panels(a_t_like):
    """[K, C] fp32 -> [128ki, C/512 panel, 16ko, 512] (k = ko*128 + ki)."""
    return np.ascontiguousarray(
        a_t_like.reshape(KO, P, -1, FD).transpose(1, 2, 0, 3)
    )


def make_in_maps(input, other):
    input = np.asarray(input, dtype=np.float32)
    other = np.asarray(other, dtype=np.float32)
    wp = _pack_panels(other)
    return [
        {
            "xp": _pack_panels(input[c * M_LOC : (c + 1) * M_LOC].T),
            "wp": wp,
        }
        for c in range(N_CORES)
    ]


def kernel(input, other):
    from concourse.bass_utils import run_bass_kernel_spmd

    if "nc" not in _PROGRAM_CACHE:
        _PROGRAM_CACHE["nc"] = build_program()
    nc = _PROGRAM_CACHE["nc"]

    in_maps = make_in_maps(input, other)
    res = run_bass_kernel_spmd(nc, in_maps, list(range(N_CORES)))
    return np.concatenate([res.results[c]["out"] for c in range(N_CORES)], axis=0)


# revision 19
# speedup vs baseline: 1.1116x; 1.0002x over previous
"""FP8 quantized matmul kernel for Trainium2 (8 NeuronCores, SPMD).

Computes: out = fp8_quant(input) @ fp8_quant(other), bf16 output.
  input: [16384, 2048] fp32, other: [2048, 2048] fp32.

Sharding: data-parallel over M. Each core processes 2048 rows of `input`
and a full replica of `other`; no cross-core communication. During
host-side sharding both operands are packed K-major into 512-wide
panel-of-column blocks ([128ki, panel, ko, 512] fp32), so every device
load is per-partition contiguous (8 KB lines, peak HBM efficiency) and
no on-device transposes are needed.

Per-core pipeline (all on device):
  1. A panels (input^T columns) and B panels (other columns) stream in
     as [128, 4ko, 512] chunks via gpsimd SWDGE DMAs that cast
     fp32 -> fp8e4m3 in flight (RNE saturating, exactly matching the
     reference quant for ~N(0,1) data where the +-448 clip never fires)
     straight into SBUF-resident qat / qb: no stage buffers and no
     Vector/Scalar quant work (lower power -> fewer PE util-throttle
     windows). Streaming is B-ahead ((A0|B0|B1), (B2|A1), (B3|A2), (A3))
     so the last-arriving panel gates only 4 output tiles.
  2. FP8 DoubleRow matmuls (K paired 2x128) accumulate fp32 in PSUM;
     tiles are emitted the moment their last operand's load is issued so
     the in-order PE queue matches data arrival (first MM at ~13us of a
     ~158us kernel; PE runs at the 222 ns/MM DoubleRow peak after ramp).
  3. PSUM evicts to bf16 on Vector (3 of 4 slices) / Scalar (last slice)
     and stores via the Scalar-engine HWDGE queue (separate from load
     issue so store waits never block loads), batched [128, 4, 512].

Measured (8-core SPMD, axon trn2, NTFF device exec time, core 0):
~158 us in unthrottled windows (~170 us when the chip power-throttles
the PE; ham type-1 k=4/n=8 windows). Baseline this session started at
~192 us (on-device PE transposes + whole-matrix-first schedule).
"""

import numpy as np

P = 128
M_LOC, K, N = 2048, 2048, 2048
N_CORES = 8
KO = K // P       # 16 k-blocks of 128
KP = KO // 2      # 8 DoubleRow k-pairs
FD = 512          # matmul free dim (one PSUM bank of fp32)
NT = N // FD      # 4 n panels
MG = M_LOC // FD  # 4 m groups (512 wide)
MI = FD // P      # 4 m slices per group
CKO = 4           # ko blocks per streamed chunk
KC = KO // CKO    # 4 k-chunks per panel/group

import os
XF_BUFS = int(os.environ.get('XF_BUFS', '3'))
WF_BUFS = int(os.environ.get('WF_BUFS', '3'))
OSB_BUFS = int(os.environ.get('OSB_BUFS', '4'))
PSUM_BUFS = int(os.environ.get('PSUM_BUFS', '8'))
OUT_ENG = os.environ.get('OUT_ENG', 'scalar')  # scalar | gpsimd | sync


def build(tc, xp, wp, out, iters=1, hw_loop=False):
    """Emit the per-core kernel IR. xp: [128, MG, KO, FD] f32 (the input
    shard, K-major panel-packed), wp: [128, NT, KO, FD] f32 (other,
    panel-packed), out: [M_LOC,N] bf16 (all DRAM APs). iters>1 repeats
    the whole computation (python-unrolled, or a hardware For_i loop when
    hw_loop=True) for marginal-time benchmarking."""
    import contextlib

    import concourse.mybir as mybir

    nc = tc.nc
    f32 = mybir.dt.float32
    bf16 = mybir.dt.bfloat16
    fp8 = mybir.dt.float8e4

    out_r = out.rearrange("(t p) n -> p t n", p=P)  # m row = t*128 + p

    with (
        tc.tile_pool(name="resident", bufs=1) as resident,
        tc.tile_pool(name="stage", bufs=4) as stage,
        tc.tile_pool(name="ostage", bufs=4) as ostage,
        tc.tile_pool(name="psum_mm", bufs=PSUM_BUFS, space="PSUM") as psum_mm,
    ):
        if hw_loop:
            loop_ctx = tc.For_i(0, iters, 1)
            reps = 1
        else:
            loop_ctx = contextlib.nullcontext()
            reps = iters

        with loop_ctx:
            for _ in range(reps):
                _emit_body(tc, xp, wp, out_r, resident, stage, ostage,
                           psum_mm, mybir, f32, bf16, fp8)


def _emit_body(tc, xp, wp, out_r, resident, stage, ostage, psum_mm,
               mybir, f32, bf16, fp8):
    nc = tc.nc

    # [ki, g, ko, m] = quant(input)^T at k = ko*128 + ki, m = g*512 + m
    qat = resident.tile([P, MG, KO, FD], fp8, tag="qat")
    # [ki, p, ko, n] = quant(other) at k = ko*128 + ki, n = p*512 + n
    qb = resident.tile([P, NT, KO, FD], fp8, tag="qb")

    QCAST = os.environ.get('QCAST', 'xw')  # chars 'x'/'w': quantize via SWDGE DMA cast

    def load_quant_chunk(src, dst, col, ko0, nko, which):
        """DMA one [128, nko, FD] f32 chunk (ko blocks [ko0, ko0+nko) of
        panel `col`, per-partition contiguous in DRAM) and quantize it
        into the fp8 resident tile. Staged in CKO-sized slots so small
        head chunks and full chunks share one pool tag."""
        ks = slice(ko0, ko0 + nko)
        if which in QCAST:
            # gpsimd SWDGE casts f32 -> fp8 in flight: no stage buffer,
            # no quant engine work.
            nc.gpsimd.dma_start(dst[:, col, ks, :], src[:, col, ks, :])
            return
        f = stage.tile(
            [P, CKO, FD], f32, tag=f"{which}f", name=f"{which}f_{col}_{ko0}",
            bufs=(XF_BUFS if which == "x" else WF_BUFS),
        )
        nc.sync.dma_start(f[:, :nko, :], src[:, col, ks, :])
        eng = nc.vector.tensor_copy if which == "x" else nc.scalar.copy
        eng(dst[:, col, ks, :], f[:, :nko, :])

    pts = {}

    def mm_half(g, p, half):
        """Split-K half of tile (g, p): kp 4*half..4*half+3 accumulate in
        PSUM; half 0 evicts an fp32 partial (freeing the bank for another
        ramp tile), half 1 adds the partial back during the bf16 evict.
        Numerically one extra fp32 add vs single-pass accumulation."""
        if half == 0:
            pts[(g, p)] = ostage.tile(
                [P, MI, FD], f32, tag="partial", name=f"pt_{g}_{p}", bufs=4
            )
        else:
            osb = ostage.tile(
                [P, MI, FD], bf16, tag="osb", name=f"osb_{g}_{p}", bufs=OSB_BUFS
            )
        pt = pts[(g, p)]
        for mi in range(MI):
            ps = psum_mm.tile(
                [P, FD], f32, tag="ps", name=f"ps_{g}_{p}_{mi}_{half}"
            )
            for kp in range(4 * half, 4 * half + 4):
                nc.tensor.matmul(
                    ps,
                    qat[:, g, 2 * kp : 2 * kp + 2, mi * P : (mi + 1) * P],
                    qb[:, p, 2 * kp : 2 * kp + 2, :],
                    start=(kp % 4 == 0),
                    stop=(kp % 4 == 3),
                    perf_mode=mybir.MatmulPerfMode.DoubleRow,
                )
            if half == 0:
                if mi % 2 == 1:
                    nc.scalar.copy(pt[:, mi, :], ps)
                else:
                    nc.vector.tensor_copy(pt[:, mi, :], ps)
            else:
                nc.vector.scalar_tensor_tensor(
                    osb[:, mi, :], ps, 0.0, pt[:, mi, :],
                    op0=mybir.AluOpType.add, op1=mybir.AluOpType.add,
                )
        if half == 1:
            out_eng = {"scalar": nc.scalar, "gpsimd": nc.gpsimd,
                       "sync": nc.sync}[OUT_ENG]
            out_eng.dma_start(
                out_r[:, g * MI : (g + 1) * MI, p * FD : (p + 1) * FD], osb
            )

    def mm_tile(g, p):
        """All MMs for output tile (m-group g, n-panel p): 4 m-slices of
        [128, 512], each accumulating 8 DoubleRow k-pairs in PSUM."""
        osb = ostage.tile(
            [P, MI, FD], bf16, tag="osb", name=f"osb_{g}_{p}", bufs=OSB_BUFS
        )
        for mi in range(MI):
            ps = psum_mm.tile([P, FD], f32, tag="ps", name=f"ps_{g}_{p}_{mi}")
            for kp in range(KP):
                nc.tensor.matmul(
                    ps,
                    qat[:, g, 2 * kp : 2 * kp + 2, mi * P : (mi + 1) * P],
                    qb[:, p, 2 * kp : 2 * kp + 2, :],
                    start=(kp == 0),
                    stop=(kp == KP - 1),
                    perf_mode=mybir.MatmulPerfMode.DoubleRow,
                )
            if mi % 2 == 1:
                nc.scalar.copy(osb[:, mi, :], ps)
            else:
                nc.vector.tensor_copy(osb[:, mi, :], ps)
        out_eng = {"scalar": nc.scalar, "gpsimd": nc.gpsimd, "sync": nc.sync}[OUT_ENG]
        out_eng.dma_start(
            out_r[:, g * MI : (g + 1) * MI, p * FD : (p + 1) * FD], osb
        )

    # k-chunk splits: small head chunks for the first pair so the tensor
    # engine starts as early as possible.
    HEAD = [(0, 2), (2, 2), (4, 4), (8, 4), (12, 4)]
    FINE = [(0, 2), (2, 2), (4, 2), (6, 2), (8, 2), (10, 2), (12, 2), (14, 2)]
    FULL = [(0, 4), (4, 4), (8, 4), (12, 4)]

    # B-ahead streaming: phase 1 round-robins (A0 | B0 | B1) so two B
    # panels land by the time A0's tiles are consumed; later phases keep
    # >=2 streams chunk-interleaved (single-stream issue loses ~20% DMA
    # bandwidth to stage-buffer stalls on the in-order sync queue).
    # Tiles are emitted the moment their last operand's load has been
    # issued, so the PE queue order matches data arrival. The
    # last-arriving panel (A3) gates only 4 tiles of PE work.
    SCHED = os.environ.get('SCHED', 'a')
    if SCHED == 'a':
        phases = [
            ([("x", 0, HEAD), ("w", 0, HEAD), ("w", 1, FULL)],
             [(0, 0), (0, 1)]),
            ([("w", 2, FULL), ("x", 1, FULL)],
             [(0, 2), (1, 0), (1, 1), (1, 2)]),
            ([("w", 3, FULL), ("x", 2, FULL)],
             [(0, 3), (1, 3), (2, 0), (2, 1), (2, 2), (2, 3)]),
            ([("x", 3, FULL)],
             [(3, 0), (3, 1), (3, 2), (3, 3)]),
        ]
    elif SCHED == 'b':  # 4-way phase 1
        phases = [
            ([("x", 0, HEAD), ("w", 0, HEAD), ("w", 1, FULL), ("w", 2, FULL)],
             [(0, 0), (0, 1), (0, 2)]),
            ([("w", 3, FULL), ("x", 1, FULL)],
             [(0, 3), (1, 0), (1, 1), (1, 2), (1, 3)]),
            ([("x", 2, FULL), ("x", 3, FULL)],
             [(2, 0), (2, 1), (2, 2), (2, 3), (3, 0), (3, 1), (3, 2), (3, 3)]),
        ]
    elif SCHED == 'f':
        # Split-K ramp: stream 4 panels 4-way in phase 1 and run all four
        # (g<2, p<2) tiles concurrently by recycling PSUM banks through
        # fp32 partial eviction; phases 2-3 are normal full-K tiles.
        phases = [
            ([("x", 0, HEAD), ("w", 0, HEAD), ("x", 1, FULL), ("w", 1, FULL)],
             [("h", 0, 0, 0), ("h", 0, 1, 0), ("h", 1, 0, 0), ("h", 1, 1, 0),
              ("h", 0, 0, 1), ("h", 0, 1, 1), ("h", 1, 0, 1), ("h", 1, 1, 1)]),
            ([("w", 2, FULL), ("x", 2, FULL)],
             [(0, 2), (1, 2), (2, 0), (2, 1), (2, 2)]),
            ([("w", 3, FULL), ("x", 3, FULL)],
             [(0, 3), (1, 3), (2, 3), (3, 0), (3, 1), (3, 2), (3, 3)]),
        ]
    elif SCHED == 'e':  # 2-way prime of the first chunks, then B-ahead
        phases = [
            ([("x", 0, HEAD[:1]), ("w", 0, HEAD[:1])], []),
            ([("x", 0, HEAD[1:]), ("w", 0, HEAD[1:]), ("w", 1, FULL)],
             [(0, 0), (0, 1)]),
            ([("w", 2, FULL), ("x", 1, FULL)],
             [(0, 2), (1, 0), (1, 1), (1, 2)]),
            ([("w", 3, FULL), ("x", 2, FULL)],
             [(0, 3), (1, 3), (2, 0), (2, 1), (2, 2), (2, 3)]),
            ([("x", 3, FULL)],
             [(3, 0), (3, 1), (3, 2), (3, 3)]),
        ]
    elif SCHED == 'd':  # fine chunks everywhere
        phases = [
            ([("x", 0, FINE), ("w", 0, FINE), ("w", 1, FINE)],
             [(0, 0), (0, 1)]),
            ([("w", 2, FINE), ("x", 1, FINE)],
             [(0, 2), (1, 0), (1, 1), (1, 2)]),
            ([("w", 3, FINE), ("x", 2, FINE)],
             [(0, 3), (1, 3), (2, 0), (2, 1), (2, 2), (2, 3)]),
            ([("x", 3, FINE)],
             [(3, 0), (3, 1), (3, 2), (3, 3)]),
        ]
    else:  # 'c': fine chunks for pair 0
        phases = [
            ([("x", 0, FINE), ("w", 0, FINE), ("w", 1, FULL)],
             [(0, 0), (0, 1)]),
            ([("w", 2, FULL), ("x", 1, FULL)],
             [(0, 2), (1, 0), (1, 1), (1, 2)]),
            ([("w", 3, FULL), ("x", 2, FULL)],
             [(0, 3), (1, 3), (2, 0), (2, 1), (2, 2), (2, 3)]),
            ([("x", 3, FULL)],
             [(3, 0), (3, 1), (3, 2), (3, 3)]),
        ]
    for loads, tiles in phases:
        nchunks = max(len(cl) for _, _, cl in loads)
        for i in range(nchunks):
            for which, panel, chunklist in loads:
                if i < len(chunklist):
                    k0, nko = chunklist[i]
                    load_quant_chunk(
                        xp if which == "x" else wp,
                        qat if which == "x" else qb,
                        panel, k0, nko, which,
                    )
        for t in tiles:
            if len(t) == 4:
                _, g, p, half = t
                mm_half(g, p, half)
            else:
                g, p = t
                mm_tile(g, p)


def build_program(iters=1):
    """Build and compile the single-core SPMD program."""
    import concourse.bacc as bacc
    import concourse.mybir as mybir
    import concourse.tile as tile

    nc = bacc.Bacc("TRN2", target_bir_lowering=False, debug=False)
    xp = nc.dram_tensor(
        "xp", [P, MG, KO, FD], mybir.dt.float32, kind="ExternalInput"
    ).ap()
    wp = nc.dram_tensor(
        "wp", [P, NT, KO, FD], mybir.dt.float32, kind="ExternalInput"
    ).ap()
    out = nc.dram_tensor(
        "out", [M_LOC, N], mybir.dt.bfloat16, kind="ExternalOutput"
    ).ap()
    with tile.TileContext(nc) as tc:
        build(tc, xp, wp, out, iters=iters)
    nc.compile()
    return nc


_PROGRAM_CACHE = {}


def _pack_panels(a_t_like):
    """[K, C] fp32 -> [128ki, C/512 panel, 16ko, 512] (k = ko*128 + ki)."""
    return np.ascontiguousarray(
        a_t_like.reshape(KO, P, -1, FD).transpose(1, 2, 0, 3)
    )


def make_in_maps(input, other):
    input = np.asarray(input, dtype=np.float32)
    other = np.asarray(other, dtype=np.float32)
    wp = _pack_panels(other)
    return [
        {
            "xp": _pack_panels(input[c * M_LOC : (c + 1) * M_LOC].T),
            "wp": wp,
        }
        for c in range(N_CORES)
    ]


def kernel(input, other):
    from concourse.bass_utils import run_bass_kernel_spmd

    if "nc" not in _PROGRAM_CACHE:
        _PROGRAM_CACHE["nc"] = build_program()
    nc = _PROGRAM_CACHE["nc"]

    in_maps = make_in_maps(input, other)
    res = run_bass_kernel_spmd(nc, in_maps, list(range(N_CORES)))
    return np.concatenate([res.results[c]["out"] for c in range(N_CORES)], axis=0)


# revision 25
# speedup vs baseline: 1.1155x; 1.0034x over previous
"""FP8 quantized matmul kernel for Trainium2 (8 NeuronCores, SPMD).

Computes: out = fp8_quant(input) @ fp8_quant(other), bf16 output.
  input: [16384, 2048] fp32, other: [2048, 2048] fp32.

Sharding: data-parallel over M. Each core processes 2048 rows of `input`
and a full replica of `other`; no cross-core communication. During
host-side sharding both operands are packed K-major into 512-wide
panel-of-column blocks ([128ki, panel, ko, 512] fp32), so every device
load is per-partition contiguous (8 KB lines, peak HBM efficiency) and
no on-device transposes are needed.

Per-core pipeline (all on device):
  1. A panels (input^T columns) and B panels (other columns) stream in
     as [128, 4ko, 512] chunks via gpsimd SWDGE DMAs that cast
     fp32 -> fp8e4m3 in flight (RNE saturating, exactly matching the
     reference quant for ~N(0,1) data where the +-448 clip never fires)
     straight into SBUF-resident qat / qb: no stage buffers and no
     Vector/Scalar quant work (lower power -> fewer PE util-throttle
     windows). Streaming is B-ahead ((A0|B0|B1), (B2|A1), (B3|A2), (A3))
     so the last-arriving panel gates only 4 output tiles.
  2. FP8 DoubleRow matmuls (K paired 2x128) accumulate fp32 in PSUM;
     tiles are emitted the moment their last operand's load is issued so
     the in-order PE queue matches data arrival (first MM at ~13us of a
     ~158us kernel; PE runs at the 222 ns/MM DoubleRow peak after ramp).
  3. PSUM evicts to bf16 on Vector (3 of 4 slices) / Scalar (last slice)
     and stores via the Scalar-engine HWDGE queue (separate from load
     issue so store waits never block loads), batched [128, 4, 512].

Measured (8-core SPMD, axon trn2, NTFF device exec time, core 0):
~158 us in unthrottled windows (~170 us when the chip power-throttles
the PE; ham type-1 k=4/n=8 windows). Baseline this session started at
~192 us (on-device PE transposes + whole-matrix-first schedule).
"""

import numpy as np

P = 128
M_LOC, K, N = 2048, 2048, 2048
N_CORES = 8
KO = K // P       # 16 k-blocks of 128
KP = KO // 2      # 8 DoubleRow k-pairs
FD = 512          # matmul free dim (one PSUM bank of fp32)
NT = N // FD      # 4 n panels
MG = M_LOC // FD  # 4 m groups (512 wide)
MI = FD // P      # 4 m slices per group
CKO = 4           # ko blocks per streamed chunk
KC = KO // CKO    # 4 k-chunks per panel/group

import os
XF_BUFS = int(os.environ.get('XF_BUFS', '3'))
WF_BUFS = int(os.environ.get('WF_BUFS', '3'))
OSB_BUFS = int(os.environ.get('OSB_BUFS', '4'))
PSUM_BUFS = int(os.environ.get('PSUM_BUFS', '8'))
OUT_ENG = os.environ.get('OUT_ENG', 'scalar')  # scalar | gpsimd | sync


def build(tc, xp, wp, out, iters=1, hw_loop=False):
    """Emit the per-core kernel IR. xp: [128, MG, KO, FD] f32 (the input
    shard, K-major panel-packed), wp: [128, NT, KO, FD] f32 (other,
    panel-packed), out: [M_LOC,N] bf16 (all DRAM APs). iters>1 repeats
    the whole computation (python-unrolled, or a hardware For_i loop when
    hw_loop=True) for marginal-time benchmarking."""
    import contextlib

    import concourse.mybir as mybir

    nc = tc.nc
    f32 = mybir.dt.float32
    bf16 = mybir.dt.bfloat16
    fp8 = mybir.dt.float8e4

    out_r = out.rearrange("(t p) n -> p t n", p=P)  # m row = t*128 + p

    with (
        tc.tile_pool(name="resident", bufs=1) as resident,
        tc.tile_pool(name="stage", bufs=4) as stage,
        tc.tile_pool(name="ostage", bufs=4) as ostage,
        tc.tile_pool(name="psum_mm", bufs=PSUM_BUFS, space="PSUM") as psum_mm,
    ):
        if hw_loop:
            loop_ctx = tc.For_i(0, iters, 1)
            reps = 1
        else:
            loop_ctx = contextlib.nullcontext()
            reps = iters

        with loop_ctx:
            for _ in range(reps):
                _emit_body(tc, xp, wp, out_r, resident, stage, ostage,
                           psum_mm, mybir, f32, bf16, fp8)


def _emit_body(tc, xp, wp, out_r, resident, stage, ostage, psum_mm,
               mybir, f32, bf16, fp8):
    nc = tc.nc

    # [ki, g, ko, m] = quant(input)^T at k = ko*128 + ki, m = g*512 + m
    qat = resident.tile([P, MG, KO, FD], fp8, tag="qat")
    # [ki, p, ko, n] = quant(other) at k = ko*128 + ki, n = p*512 + n
    qb = resident.tile([P, NT, KO, FD], fp8, tag="qb")

    QCAST = os.environ.get('QCAST', 'xw')  # chars 'x'/'w': quantize via SWDGE DMA cast

    def load_quant_chunk(src, dst, col, ko0, nko, which, hwdge=False):
        """DMA one [128, nko, FD] f32 chunk (ko blocks [ko0, ko0+nko) of
        panel `col`, per-partition contiguous in DRAM) and quantize it
        into the fp8 resident tile. Staged in CKO-sized slots so small
        head chunks and full chunks share one pool tag. hwdge=True forces
        the sync-engine HWDGE + engine-quant path (starts ~10us before
        the SWDGE ring warms up — use for the first panels)."""
        ks = slice(ko0, ko0 + nko)
        if which in QCAST and not hwdge:
            # gpsimd SWDGE casts f32 -> fp8 in flight: no stage buffer,
            # no quant engine work.
            nc.gpsimd.dma_start(dst[:, col, ks, :], src[:, col, ks, :])
            return
        f = stage.tile(
            [P, CKO, FD], f32, tag=f"{which}f", name=f"{which}f_{col}_{ko0}",
            bufs=(XF_BUFS if which == "x" else WF_BUFS),
        )
        nc.sync.dma_start(f[:, :nko, :], src[:, col, ks, :])
        eng = nc.vector.tensor_copy if which == "x" else nc.scalar.copy
        eng(dst[:, col, ks, :], f[:, :nko, :])

    pts = {}

    def mm_half(g, p, half):
        """Split-K half of tile (g, p): kp 4*half..4*half+3 accumulate in
        PSUM; half 0 evicts an fp32 partial (freeing the bank for another
        ramp tile), half 1 adds the partial back during the bf16 evict.
        Numerically one extra fp32 add vs single-pass accumulation."""
        if half == 0:
            pts[(g, p)] = ostage.tile(
                [P, MI, FD], f32, tag="partial", name=f"pt_{g}_{p}", bufs=4
            )
        else:
            osb = ostage.tile(
                [P, MI, FD], bf16, tag="osb", name=f"osb_{g}_{p}", bufs=OSB_BUFS
            )
        pt = pts[(g, p)]
        for mi in range(MI):
            ps = psum_mm.tile(
                [P, FD], f32, tag="ps", name=f"ps_{g}_{p}_{mi}_{half}"
            )
            for kp in range(4 * half, 4 * half + 4):
                nc.tensor.matmul(
                    ps,
                    qat[:, g, 2 * kp : 2 * kp + 2, mi * P : (mi + 1) * P],
                    qb[:, p, 2 * kp : 2 * kp + 2, :],
                    start=(kp % 4 == 0),
                    stop=(kp % 4 == 3),
                    perf_mode=mybir.MatmulPerfMode.DoubleRow,
                )
            if half == 0:
                if mi % 2 == 1:
                    nc.scalar.copy(pt[:, mi, :], ps)
                else:
                    nc.vector.tensor_copy(pt[:, mi, :], ps)
            else:
                nc.vector.scalar_tensor_tensor(
                    osb[:, mi, :], ps, 0.0, pt[:, mi, :],
                    op0=mybir.AluOpType.add, op1=mybir.AluOpType.add,
                )
        if half == 1:
            out_eng = {"scalar": nc.scalar, "gpsimd": nc.gpsimd,
                       "sync": nc.sync}[OUT_ENG]
            out_eng.dma_start(
                out_r[:, g * MI : (g + 1) * MI, p * FD : (p + 1) * FD], osb
            )

    def mm_tiles_chase(tiles):
        """Emit a group of tiles (<=2, 8 PSUM slices) kp-major so the PE
        consumes each arriving k-chunk across all slices immediately
        instead of head-of-line blocking on one slice's future chunks."""
        oss, pss = {}, {}
        for g, p in tiles:
            oss[g, p] = ostage.tile(
                [P, MI, FD], bf16, tag="osb", name=f"osb_{g}_{p}",
                bufs=OSB_BUFS,
            )
            for mi in range(MI):
                pss[g, p, mi] = psum_mm.tile(
                    [P, FD], f32, tag="ps", name=f"ps_{g}_{p}_{mi}"
                )
        for kp in range(KP):
            for g, p in tiles:
                for mi in range(MI):
                    nc.tensor.matmul(
                        pss[g, p, mi],
                        qat[:, g, 2 * kp : 2 * kp + 2, mi * P : (mi + 1) * P],
                        qb[:, p, 2 * kp : 2 * kp + 2, :],
                        start=(kp == 0),
                        stop=(kp == KP - 1),
                        perf_mode=mybir.MatmulPerfMode.DoubleRow,
                    )
        for g, p in tiles:
            for mi in range(MI):
                if mi % 2 == 1:
                    nc.scalar.copy(oss[g, p][:, mi, :], pss[g, p, mi])
                else:
                    nc.vector.tensor_copy(oss[g, p][:, mi, :], pss[g, p, mi])
            out_eng = {"scalar": nc.scalar, "gpsimd": nc.gpsimd,
                       "sync": nc.sync}[OUT_ENG]
            out_eng.dma_start(
                out_r[:, g * MI : (g + 1) * MI, p * FD : (p + 1) * FD],
                oss[g, p],
            )

    def mm_tile(g, p):
        """All MMs for output tile (m-group g, n-panel p): 4 m-slices of
        [128, 512], each accumulating 8 DoubleRow k-pairs in PSUM."""
        osb = ostage.tile(
            [P, MI, FD], bf16, tag="osb", name=f"osb_{g}_{p}", bufs=OSB_BUFS
        )
        for mi in range(MI):
            ps = psum_mm.tile([P, FD], f32, tag="ps", name=f"ps_{g}_{p}_{mi}")
            for kp in range(KP):
                nc.tensor.matmul(
                    ps,
                    qat[:, g, 2 * kp : 2 * kp + 2, mi * P : (mi + 1) * P],
                    qb[:, p, 2 * kp : 2 * kp + 2, :],
                    start=(kp == 0),
                    stop=(kp == KP - 1),
                    perf_mode=mybir.MatmulPerfMode.DoubleRow,
                )
            if mi % 2 == 1:
                nc.scalar.copy(osb[:, mi, :], ps)
            else:
                nc.vector.tensor_copy(osb[:, mi, :], ps)
        out_eng = {"scalar": nc.scalar, "gpsimd": nc.gpsimd, "sync": nc.sync}[OUT_ENG]
        out_eng.dma_start(
            out_r[:, g * MI : (g + 1) * MI, p * FD : (p + 1) * FD], osb
        )

    # k-chunk splits: small head chunks for the first pair so the tensor
    # engine starts as early as possible.
    HEAD = [(0, 2), (2, 2), (4, 4), (8, 4), (12, 4)]
    FINE = [(0, 2), (2, 2), (4, 2), (6, 2), (8, 2), (10, 2), (12, 2), (14, 2)]
    FULL = [(0, 4), (4, 4), (8, 4), (12, 4)]

    # B-ahead streaming: phase 1 round-robins (A0 | B0 | B1) so two B
    # panels land by the time A0's tiles are consumed; later phases keep
    # >=2 streams chunk-interleaved (single-stream issue loses ~20% DMA
    # bandwidth to stage-buffer stalls on the in-order sync queue).
    # Tiles are emitted the moment their last operand's load has been
    # issued, so the PE queue order matches data arrival. The
    # last-arriving panel (A3) gates only 4 tiles of PE work.
    SCHED = os.environ.get('SCHED', 'a')
    if SCHED == 'a':
        phases = [
            ([("x", 0, HEAD), ("w", 0, HEAD), ("w", 1, FULL)],
             [(0, 0), (0, 1)]),
            ([("w", 2, FULL), ("x", 1, FULL)],
             [(0, 2), (1, 0), (1, 1), (1, 2)]),
            ([("w", 3, FULL), ("x", 2, FULL)],
             [(0, 3), (1, 3), (2, 0), (2, 1), (2, 2), (2, 3)]),
            ([("x", 3, FULL)],
             [(3, 0), (3, 1), (3, 2), (3, 3)]),
        ]
    elif SCHED == 'i':
        # Like 'a' but the phase-1 ramp tiles are emitted kp-major
        # (chunk-chasing) so PE work tracks chunk arrival.
        phases = [
            ([("x", 0, HEAD), ("w", 0, HEAD), ("w", 1, FULL)],
             [("chase", (0, 0), (0, 1))]),
            ([("w", 2, FULL), ("x", 1, FULL)],
             [(0, 2), (1, 0), (1, 1), (1, 2)]),
            ([("w", 3, FULL), ("x", 2, FULL)],
             [(0, 3), (1, 3), (2, 0), (2, 1), (2, 2), (2, 3)]),
            ([("x", 3, FULL)],
             [(3, 0), (3, 1), (3, 2), (3, 3)]),
        ]
    elif SCHED == 'h':
        # Hybrid: phase-1 panels ride the HWDGE+engine-quant path (fast
        # start while the SWDGE ring warms up, quant engines idle then
        # anyway); later panels use SWDGE cast-in-flight.
        phases = [
            ([("x", 0, HEAD, True), ("w", 0, HEAD, True), ("w", 1, FULL, True)],
             [(0, 0), (0, 1)]),
            ([("w", 2, FULL), ("x", 1, FULL)],
             [(0, 2), (1, 0), (1, 1), (1, 2)]),
            ([("w", 3, FULL), ("x", 2, FULL)],
             [(0, 3), (1, 3), (2, 0), (2, 1), (2, 2), (2, 3)]),
            ([("x", 3, FULL)],
             [(3, 0), (3, 1), (3, 2), (3, 3)]),
        ]
    elif SCHED == 'b':  # 4-way phase 1
        phases = [
            ([("x", 0, HEAD), ("w", 0, HEAD), ("w", 1, FULL), ("w", 2, FULL)],
             [(0, 0), (0, 1), (0, 2)]),
            ([("w", 3, FULL), ("x", 1, FULL)],
             [(0, 3), (1, 0), (1, 1), (1, 2), (1, 3)]),
            ([("x", 2, FULL), ("x", 3, FULL)],
             [(2, 0), (2, 1), (2, 2), (2, 3), (3, 0), (3, 1), (3, 2), (3, 3)]),
        ]
    elif SCHED == 'f':
        # Split-K ramp: stream 4 panels 4-way in phase 1 and run all four
        # (g<2, p<2) tiles concurrently by recycling PSUM banks through
        # fp32 partial eviction; phases 2-3 are normal full-K tiles.
        phases = [
            ([("x", 0, HEAD), ("w", 0, HEAD), ("x", 1, FULL), ("w", 1, FULL)],
             [("h", 0, 0, 0), ("h", 0, 1, 0), ("h", 1, 0, 0), ("h", 1, 1, 0),
              ("h", 0, 0, 1), ("h", 0, 1, 1), ("h", 1, 0, 1), ("h", 1, 1, 1)]),
            ([("w", 2, FULL), ("x", 2, FULL)],
             [(0, 2), (1, 2), (2, 0), (2, 1), (2, 2)]),
            ([("w", 3, FULL), ("x", 3, FULL)],
             [(0, 3), (1, 3), (2, 3), (3, 0), (3, 1), (3, 2), (3, 3)]),
        ]
    elif SCHED == 'e':  # 2-way prime of the first chunks, then B-ahead
        phases = [
            ([("x", 0, HEAD[:1]), ("w", 0, HEAD[:1])], []),
            ([("x", 0, HEAD[1:]), ("w", 0, HEAD[1:]), ("w", 1, FULL)],
             [(0, 0), (0, 1)]),
            ([("w", 2, FULL), ("x", 1, FULL)],
             [(0, 2), (1, 0), (1, 1), (1, 2)]),
            ([("w", 3, FULL), ("x", 2, FULL)],
             [(0, 3), (1, 3), (2, 0), (2, 1), (2, 2), (2, 3)]),
            ([("x", 3, FULL)],
             [(3, 0), (3, 1), (3, 2), (3, 3)]),
        ]
    elif SCHED == 'd':  # fine chunks everywhere
        phases = [
            ([("x", 0, FINE), ("w", 0, FINE), ("w", 1, FINE)],
             [(0, 0), (0, 1)]),
            ([("w", 2, FINE), ("x", 1, FINE)],
             [(0, 2), (1, 0), (1, 1), (1, 2)]),
            ([("w", 3, FINE), ("x", 2, FINE)],
             [(0, 3), (1, 3), (2, 0), (2, 1), (2, 2), (2, 3)]),
            ([("x", 3, FINE)],
             [(3, 0), (3, 1), (3, 2), (3, 3)]),
        ]
    else:  # 'c': fine chunks for pair 0
        phases = [
            ([("x", 0, FINE), ("w", 0, FINE), ("w", 1, FULL)],
             [(0, 0), (0, 1)]),
            ([("w", 2, FULL), ("x", 1, FULL)],
             [(0, 2), (1, 0), (1, 1), (1, 2)]),
            ([("w", 3, FULL), ("x", 2, FULL)],
             [(0, 3), (1, 3), (2, 0), (2, 1), (2, 2), (2, 3)]),
            ([("x", 3, FULL)],
             [(3, 0), (3, 1), (3, 2), (3, 3)]),
        ]
    for loads, tiles in phases:
        nchunks = max(len(spec[2]) for spec in loads)
        for i in range(nchunks):
            for spec in loads:
                which, panel, chunklist = spec[0], spec[1], spec[2]
                hwdge = spec[3] if len(spec) > 3 else False
                if i < len(chunklist):
                    k0, nko = chunklist[i]
                    load_quant_chunk(
                        xp if which == "x" else wp,
                        qat if which == "x" else qb,
                        panel, k0, nko, which, hwdge=hwdge,
                    )
        for t in tiles:
            if t[0] == "chase":
                mm_tiles_chase(list(t[1:]))
            elif len(t) == 4:
                _, g, p, half = t
                mm_half(g, p, half)
            else:
                g, p = t
                mm_tile(g, p)


def build_program(iters=1):
    """Build and compile the single-core SPMD program."""
    import concourse.bacc as bacc
    import concourse.mybir as mybir
    import concourse.tile as tile

    nc = bacc.Bacc("TRN2", target_bir_lowering=False, debug=False)
    xp = nc.dram_tensor(
        "xp", [P, MG, KO, FD], mybir.dt.float32, kind="ExternalInput"
    ).ap()
    wp = nc.dram_tensor(
        "wp", [P, NT, KO, FD], mybir.dt.float32, kind="ExternalInput"
    ).ap()
    out = nc.dram_tensor(
        "out", [M_LOC, N], mybir.dt.bfloat16, kind="ExternalOutput"
    ).ap()
    with tile.TileContext(nc) as tc:
        build(tc, xp, wp, out, iters=iters)
    nc.compile()
    return nc


_PROGRAM_CACHE = {}


def _pack_panels(a_t_like):
    """[K, C] fp32 -> [128ki, C/512 panel, 16ko, 512] (k = ko*128 + ki)."""
    return np.ascontiguousarray(
        a_t_like.reshape(KO, P, -1, FD).transpose(1, 2, 0, 3)
    )


def make_in_maps(input, other):
    input = np.asarray(input, dtype=np.float32)
    other = np.asarray(other, dtype=np.float32)
    wp = _pack_panels(other)
    return [
        {
            "xp": _pack_panels(input[c * M_LOC : (c + 1) * M_LOC].T),
            "wp": wp,
        }
        for c in range(N_CORES)
    ]


def kernel(input, other):
    from concourse.bass_utils import run_bass_kernel_spmd

    if "nc" not in _PROGRAM_CACHE:
        _PROGRAM_CACHE["nc"] = build_program()
    nc = _PROGRAM_CACHE["nc"]

    in_maps = make_in_maps(input, other)
    res = run_bass_kernel_spmd(nc, in_maps, list(range(N_CORES)))
    return np.concatenate([res.results[c]["out"] for c in range(N_CORES)], axis=0)
